# revision 19
# baseline (speedup 1.0000x reference)
"""Trainium2 Bass kernel for nn_AttentionDecoder (single-step GRU attention
decoder, batch 1). 8-core tensor-parallel:

- attention linear sharded over L (output), logits all-gathered, softmax
  redundant on every core
- attn_applied: encoder_outputs sharded column-wise (over H)
- attn_combine sharded over the *contraction* dim -> partial [1,H] summed
  with an AllReduce, then bias+relu redundant
- GRU: w_ih / w_hh sharded over gate-output rows (each core owns its H/8
  slice of r,z,n), h_new chunks all-gathered
- out Linear sharded over vocab; per-core exp-sums all-gathered and combined
  for the log_softmax normalizer

All weight matrices are pre-transposed/packed on the host into the layout
the TensorEngine streams directly (per-partition-contiguous rhs tiles), so
every device DMA is a plain contiguous [128, F] transfer.
"""
import numpy as np

H = 2048
V = 50257
L = 512
NC = 8
HC = H // NC           # 256
LC = L // NC           # 64
KH = H // 128          # 16 k-chunks over H contraction
K2H = (2 * H) // 128   # 32 k-chunks over 2H contraction
KL = L // 128          # 4 k-chunks over L contraction
VN = 256               # vocab columns per matmul/psum chunk
NCHUNK = 25            # v-chunks per core
VC = VN * NCHUNK       # 6400 padded vocab per core
VREAL = [6283] * 7 + [V - 6283 * 7]   # real columns per core (6283/6276)
PAD_TOTAL = float(NC * VC - V)        # 943 zero-logit pad columns overall

_CACHE = {}
TRACE = False
LAST = {}


def _pack_rhs(wT):
    """[K, N] (K % 128 == 0) -> [128, (K//128)*N]; tile[p, k*N+n] = wT[k*128+p, n]."""
    K, N = wT.shape
    kk = K // 128
    return np.ascontiguousarray(
        wT.reshape(kk, 128, N).transpose(1, 0, 2).reshape(128, kk * N)
    )


def _vec_T(v):
    """[n*128] -> [128, n] with out[p, k] = v[k*128 + p]."""
    n = v.shape[0] // 128
    return np.ascontiguousarray(v.reshape(n, 128).T)


def _build():
    import concourse.bacc as bacc
    import concourse.mybir as mybir
    import concourse.tile as tile

    f32 = mybir.dt.float32
    nc = bacc.Bacc("TRN2", target_bir_lowering=False, debug=False, num_devices=NC)

    # ---- I/O ------------------------------------------------------------
    i_cehT = nc.dram_tensor("cehT", [128, K2H], f32, kind="ExternalInput")
    i_hT = nc.dram_tensor("hT", [128, KH], f32, kind="ExternalInput")
    i_hchunk = nc.dram_tensor("hchunk", [1, HC], f32, kind="ExternalInput")
    i_embT2 = nc.dram_tensor("embT2", [128, HC // 128], f32, kind="ExternalInput")
    i_enc = nc.dram_tensor("encp", [128, KL * HC], f32, kind="ExternalInput")
    i_attnw = nc.dram_tensor("attnp", [128, K2H * LC], f32, kind="ExternalInput")
    i_attnb = nc.dram_tensor("attnb", [1, LC], f32, kind="ExternalInput")
    i_comb = nc.dram_tensor("combp", [128, 4 * H], f32, kind="ExternalInput")
    i_combbT = nc.dram_tensor("combbT", [128, KH], f32, kind="ExternalInput")
    i_wih = nc.dram_tensor("wihp", [128, KH * 3 * HC], f32, kind="ExternalInput")
    i_whh = nc.dram_tensor("whhp", [128, KH * 3 * HC], f32, kind="ExternalInput")
    i_bih = nc.dram_tensor("bih", [1, 3 * HC], f32, kind="ExternalInput")
    i_bhh = nc.dram_tensor("bhh", [1, 3 * HC], f32, kind="ExternalInput")
    i_outw = nc.dram_tensor("outwp", [NCHUNK, 128, KH * VN], f32, kind="ExternalInput")
    i_outb = nc.dram_tensor("outbp", [NCHUNK, 1, VN], f32, kind="ExternalInput")
    i_ident = nc.dram_tensor("ident", [128, 128], f32, kind="ExternalInput")

    o_logits = nc.dram_tensor("o_logits", [1, VC], f32, kind="ExternalOutput")
    o_hidden = nc.dram_tensor("o_hidden", [1, H], f32, kind="ExternalOutput")
    o_attnw = nc.dram_tensor("o_attnw", [1, L], f32, kind="ExternalOutput")

    # ---- internal DRAM for collectives (all 2-D: 1-D/row DMA APs hang) --
    cc_attn_in = nc.dram_tensor("cc_attn_in", [1, LC], f32)
    cc_attn_out = nc.dram_tensor("cc_attn_out", [NC, LC], f32, addr_space="Shared")
    aw_dram = nc.dram_tensor("aw_dram", [1, L], f32)
    aa_dram = nc.dram_tensor("aa_dram", [1, HC], f32)
    cc_x_in = nc.dram_tensor("cc_x_in", [1, H], f32)
    cc_x_out = nc.dram_tensor("cc_x_out", [1, H], f32, addr_space="Shared")
    cc_h_in = nc.dram_tensor("cc_h_in", [1, HC], f32)
    cc_h_out = nc.dram_tensor("cc_h_out", [NC, HC], f32, addr_space="Shared")
    cc_s_in = nc.dram_tensor("cc_s_in", [1, 1], f32)
    cc_s_out = nc.dram_tensor("cc_s_out", [NC, 1], f32, addr_space="Shared")

    AF = mybir.ActivationFunctionType
    ALU = mybir.AluOpType
    rg = [list(range(NC))]

    with tile.TileContext(nc) as tc:
        with (
            tc.tile_pool(name="const", bufs=1) as cpool,
            tc.tile_pool(name="wts", bufs=1) as wpool,
            tc.tile_pool(name="gruw", bufs=2) as gpool,
            tc.tile_pool(name="stream", bufs=3) as spool,
            tc.tile_pool(name="small", bufs=2) as smpool,
            tc.tile_pool(name="ps", bufs=1, space="PSUM") as ps,
        ):
            # ---------- constant / small loads ----------
            ident = cpool.tile([128, 128], f32, tag="ident")
            nc.sync.dma_start(ident[:], i_ident[:])
            cehT = cpool.tile([128, K2H], f32, tag="cehT")
            nc.sync.dma_start(cehT[:], i_cehT[:])
            hT = cpool.tile([128, KH], f32, tag="hT")
            nc.sync.dma_start(hT[:], i_hT[:])
            embT2 = cpool.tile([128, HC // 128], f32, tag="embT2")
            nc.sync.dma_start(embT2[:], i_embT2[:])
            combbT = cpool.tile([128, KH], f32, tag="combbT")
            nc.sync.dma_start(combbT[:], i_combbT[:])
            hchunk = cpool.tile([1, HC], f32, tag="hchunk")
            nc.sync.dma_start(hchunk[:], i_hchunk[:])
            attnb = cpool.tile([1, LC], f32, tag="attnb")
            nc.sync.dma_start(attnb[:], i_attnb[:])
            bih = cpool.tile([1, 3 * HC], f32, tag="bih")
            nc.sync.dma_start(bih[:], i_bih[:])
            bhh = cpool.tile([1, 3 * HC], f32, tag="bhh")
            nc.sync.dma_start(bhh[:], i_bhh[:])

            # ---------- stage A: attention logits + AG + softmax ----------
            attnp = wpool.tile([128, K2H * LC], f32, tag="attnp")
            nc.sync.dma_start(attnp[:], i_attnw[:])
            p_attn = ps.tile([1, 512], f32, tag="big", bufs=2)
            for k in range(K2H):
                nc.tensor.matmul(
                    p_attn[0:1, 0:LC],
                    cehT[:, k : k + 1],
                    attnp[:, k * LC : (k + 1) * LC],
                    start=(k == 0),
                    stop=(k == K2H - 1),
                )
            al_c = cpool.tile([1, LC], f32, tag="al_c")
            nc.vector.tensor_add(al_c[:], p_attn[0:1, 0:LC], attnb[:])
            nc.sync.dma_start(cc_attn_in[:], al_c[:])
            nc.gpsimd.collective_compute(
                "AllGather", ALU.bypass, replica_groups=rg,
                ins=[cc_attn_in[:].opt()], outs=[cc_attn_out[:].opt()],
            )
            al_full = cpool.tile([1, L], f32, tag="al_full")
            nc.sync.dma_start(al_full[:], cc_attn_out[:].rearrange("a b -> (a b)")[None, :])
            # softmax over [1, L] (logits are O(1): no max subtraction needed)
            s_aw = cpool.tile([1, 1], f32, tag="s_aw")
            aw = cpool.tile([1, L], f32, tag="aw")
            nc.scalar.activation(aw[:], al_full[:], AF.Exp, accum_out=s_aw[:])
            sinv = cpool.tile([1, 1], f32, tag="sinv")
            nc.vector.reciprocal(sinv[:], s_aw[:])
            nc.vector.tensor_scalar_mul(aw[:], aw[:], sinv[0:1, 0:1])
            nc.sync.dma_start(o_attnw[:], aw[:])
            # aw -> lhsT [128, KL] via dram roundtrip + PE transpose
            nc.sync.dma_start(aw_dram[:], aw[:])
            awKL = cpool.tile([KL, 128], f32, tag="awKL")
            nc.sync.dma_start(awKL[:], aw_dram[:].rearrange("a (b c) -> (a b) c", b=KL))
            p_awT = ps.tile([128, KL], f32, tag="tr")
            nc.tensor.transpose(p_awT[:], awKL[:], ident[0:KL, 0:KL])
            awT = cpool.tile([128, KL], f32, tag="awT")
            nc.vector.tensor_copy(awT[:], p_awT[:])

            # ---------- stage B: attn_applied chunk + comb lhsT ----------
            encp = wpool.tile([128, KL * HC], f32, tag="encp")
            nc.sync.dma_start(encp[:], i_enc[:])
            p_aa = ps.tile([1, 512], f32, tag="big", bufs=2)
            for k in range(KL):
                nc.tensor.matmul(
                    p_aa[0:1, 0:HC],
                    awT[:, k : k + 1],
                    encp[:, k * HC : (k + 1) * HC],
                    start=(k == 0),
                    stop=(k == KL - 1),
                )
            aa = cpool.tile([1, HC], f32, tag="aa")
            nc.vector.tensor_copy(aa[:], p_aa[0:1, 0:HC])
            nc.sync.dma_start(aa_dram[:], aa[:])
            aa2 = cpool.tile([HC // 128, 128], f32, tag="aa2")
            nc.sync.dma_start(
                aa2[:], aa_dram[:].rearrange("a (b c) -> (a b) c", b=HC // 128)
            )
            comb_in_T = cpool.tile([128, 2 * (HC // 128)], f32, tag="comb_in_T")
            p_aaT = ps.tile([128, HC // 128], f32, tag="tr")
            nc.tensor.transpose(p_aaT[:], aa2[:], ident[0 : HC // 128, 0 : HC // 128])
            nc.vector.tensor_copy(comb_in_T[:, 0 : HC // 128], embT2[:])
            nc.vector.tensor_copy(comb_in_T[:, HC // 128 :], p_aaT[:])

            # ---------- stage C: comb partial + AllReduce + relu ----------
            combp = wpool.tile([128, 4 * H], f32, tag="combp")
            nc.sync.dma_start(combp[:], i_comb[:])
            x_part = cpool.tile([1, H], f32, tag="x_part")
            KC = 2 * HC // 128  # 4 k-chunks over the 512-dim contraction
            for q in range(H // 512):   # four 512-wide output slices
                p_x = ps.tile([1, 512], f32, tag="big", bufs=2)
                for k in range(KC):
                    nc.tensor.matmul(
                        p_x[:],
                        comb_in_T[:, k : k + 1],
                        combp[:, k * H + q * 512 : k * H + (q + 1) * 512],
                        start=(k == 0),
                        stop=(k == KC - 1),
                    )
                nc.vector.tensor_copy(x_part[0:1, q * 512 : (q + 1) * 512], p_x[:])
            nc.sync.dma_start(cc_x_in[:], x_part[:])
            nc.gpsimd.collective_compute(
                "AllReduce", ALU.add, replica_groups=rg,
                ins=[cc_x_in[:].opt()], outs=[cc_x_out[:].opt()],
            )
            x16 = cpool.tile([KH, 128], f32, tag="x16")
            nc.sync.dma_start(
                x16[:], cc_x_out[:].rearrange("a (b c) -> (a b) c", b=KH)
            )
            p_xT = ps.tile([128, KH], f32, tag="tr")
            nc.tensor.transpose(p_xT[:], x16[:], ident[0:KH, 0:KH])
            xsum = cpool.tile([128, KH], f32, tag="xsum")
            nc.vector.tensor_add(xsum[:], p_xT[:], combbT[:])
            xT = cpool.tile([128, KH], f32, tag="xT")
            nc.scalar.activation(xT[:], xsum[:], AF.Relu)

            # ---------- stage E: GRU gates ----------
            G = 3 * HC  # 768
            p_gi = ps.tile([1, G], f32, tag="gi")
            p_gh = ps.tile([1, G], f32, tag="gh")
            quarters = [(i * (KH // 4), (i + 1) * (KH // 4)) for i in range(4)]
            for (name, pk, lhsT, w_in) in (
                ("hh", p_gh, hT, i_whh),
                ("ih", p_gi, xT, i_wih),
            ):
                for h0, h1 in quarters:
                    wt = gpool.tile([128, (KH // 4) * G], f32, tag="gruw")
                    nc.sync.dma_start(
                        wt[:], w_in[:, h0 * G : h1 * G]
                    )
                    for k in range(h0, h1):
                        off = (k - h0) * G
                        for n0, n1 in ((0, 512), (512, G)):
                            nc.tensor.matmul(
                                pk[0:1, n0:n1],
                                lhsT[:, k : k + 1],
                                wt[:, off + n0 : off + n1],
                                start=(k == 0),
                                stop=(k == KH - 1),
                            )
            gi = cpool.tile([1, G], f32, tag="gi")
            nc.vector.tensor_add(gi[:], p_gi[:], bih[:])
            gh = cpool.tile([1, G], f32, tag="gh")
            nc.vector.tensor_add(gh[:], p_gh[:], bhh[:])
            t_r = cpool.tile([1, HC], f32, tag="t_r")
            nc.vector.tensor_add(t_r[:], gi[0:1, 0:HC], gh[0:1, 0:HC])
            r_g = cpool.tile([1, HC], f32, tag="r_g")
            nc.scalar.activation(r_g[:], t_r[:], AF.Sigmoid)
            t_z = cpool.tile([1, HC], f32, tag="t_z")
            nc.vector.tensor_add(t_z[:], gi[0:1, HC : 2 * HC], gh[0:1, HC : 2 * HC])
            z_g = cpool.tile([1, HC], f32, tag="z_g")
            nc.scalar.activation(z_g[:], t_z[:], AF.Sigmoid)
            t_n = cpool.tile([1, HC], f32, tag="t_n")
            nc.vector.tensor_mul(t_n[:], r_g[:], gh[0:1, 2 * HC : 3 * HC])
            t_n2 = cpool.tile([1, HC], f32, tag="t_n2")
            nc.vector.tensor_add(t_n2[:], t_n[:], gi[0:1, 2 * HC : 3 * HC])
            n_g = cpool.tile([1, HC], f32, tag="n_g")
            nc.scalar.activation(n_g[:], t_n2[:], AF.Tanh)
            # h_new = n + z * (h - n)
            t_d = cpool.tile([1, HC], f32, tag="t_d")
            nc.vector.tensor_sub(t_d[:], hchunk[:], n_g[:])
            t_e = cpool.tile([1, HC], f32, tag="t_e")
            nc.vector.tensor_mul(t_e[:], z_g[:], t_d[:])
            hn = cpool.tile([1, HC], f32, tag="hn")
            nc.vector.tensor_add(hn[:], n_g[:], t_e[:])
            nc.sync.dma_start(cc_h_in[:], hn[:])
            nc.gpsimd.collective_compute(
                "AllGather", ALU.bypass, replica_groups=rg,
                ins=[cc_h_in[:].opt()], outs=[cc_h_out[:].opt()],
            )
            nc.sync.dma_start(
                o_hidden[:], cc_h_out[:].rearrange("a b -> (a b)")[None, :]
            )
            h16 = cpool.tile([KH, 128], f32, tag="h16")
            nc.sync.dma_start(
                h16[:], cc_h_out[:].rearrange("a (b c) -> (a b) c", b=KH // NC)
            )
            p_hT = ps.tile([128, KH], f32, tag="tr")
            nc.tensor.transpose(p_hT[:], h16[:], ident[0:KH, 0:KH])
            hnT = cpool.tile([128, KH], f32, tag="hnT")
            nc.vector.tensor_copy(hnT[:], p_hT[:])

            # ---------- stage F: vocab GEMV stream ----------
            logits = cpool.tile([1, VC], f32, tag="logits")
            sums = cpool.tile([1, NCHUNK], f32, tag="sums")
            expscr = cpool.tile([1, VN], f32, tag="expscr")
            for c in range(NCHUNK):
                wt = spool.tile([128, KH * VN], f32, tag="outw")
                nc.sync.dma_start(wt[:], i_outw[c])
                ob = smpool.tile([1, VN], f32, tag="outb")
                nc.sync.dma_start(ob[:], i_outb[c])
                p_l = ps.tile([1, VN], f32, tag="big", bufs=2)
                for k in range(KH):
                    nc.tensor.matmul(
                        p_l[:],
                        hnT[:, k : k + 1],
                        wt[:, k * VN : (k + 1) * VN],
                        start=(k == 0),
                        stop=(k == KH - 1),
                    )
                nc.vector.tensor_add(logits[0:1, c * VN : (c + 1) * VN], p_l[:], ob[:])
                nc.scalar.activation(
                    expscr[:],
                    logits[0:1, c * VN : (c + 1) * VN],
                    AF.Exp,
                    accum_out=sums[0:1, c : c + 1],
                )

            # ---------- normalizer ----------
            s_loc = cpool.tile([1, 1], f32, tag="s_loc")
            nc.vector.reduce_sum(s_loc[:], sums[:], axis=mybir.AxisListType.X)
            nc.sync.dma_start(cc_s_in[:], s_loc[:])
            nc.gpsimd.collective_compute(
                "AllGather", ALU.bypass, replica_groups=rg,
                ins=[cc_s_in[:].opt()], outs=[cc_s_out[:].opt()],
            )
            s8 = cpool.tile([1, NC], f32, tag="s8")
            nc.sync.dma_start(s8[:], cc_s_out[:].rearrange("a b -> (a b)")[None, :])
            s_tot = cpool.tile([1, 1], f32, tag="s_tot")
            nc.vector.reduce_sum(s_tot[:], s8[:], axis=mybir.AxisListType.X)
            s_corr = cpool.tile([1, 1], f32, tag="s_corr")
            nc.vector.tensor_scalar_add(s_corr[:], s_tot[:], -PAD_TOTAL)
            logS = cpool.tile([1, 1], f32, tag="logS")
            nc.scalar.activation(logS[:], s_corr[:], AF.Ln)
            nc.vector.tensor_scalar(
                logits[:], logits[:], logS[0:1, 0:1], None, ALU.subtract
            )
            nc.sync.dma_start(o_logits[:], logits[:])

    nc.compile()
    return nc


def _prepare_inputs(inputs):
    token = int(np.asarray(inputs["token"]).ravel()[0])
    hidden = np.asarray(inputs["hidden"], dtype=np.float32).reshape(H)
    enc = np.asarray(inputs["encoder_outputs"], dtype=np.float32)
    emb = np.asarray(inputs["embedding"], dtype=np.float32)[token]
    attn_W = np.asarray(inputs["attn_W"], dtype=np.float32)
    attn_b = np.asarray(inputs["attn_b"], dtype=np.float32)
    comb_W = np.asarray(inputs["comb_W"], dtype=np.float32)
    comb_b = np.asarray(inputs["comb_b"], dtype=np.float32)
    w_ih = np.asarray(inputs["w_ih"], dtype=np.float32)
    w_hh = np.asarray(inputs["w_hh"], dtype=np.float32)
    b_ih = np.asarray(inputs["b_ih"], dtype=np.float32)
    b_hh = np.asarray(inputs["b_hh"], dtype=np.float32)
    out_W = np.asarray(inputs["out_W"], dtype=np.float32)
    out_b = np.asarray(inputs["out_b"], dtype=np.float32)

    ceh = np.concatenate([emb, hidden])            # [2H]
    cehT = _vec_T(ceh)                             # [128, 32]
    hT = _vec_T(hidden)                            # [128, 16]
    combbT = _vec_T(comb_b)                        # [128, 16]
    ident = np.eye(128, dtype=np.float32)
    attn_WT = np.ascontiguousarray(attn_W.T)       # [2H, L]

    in_maps = []
    for i in range(NC):
        sl_h = slice(i * HC, (i + 1) * HC)
        sl_l = slice(i * LC, (i + 1) * LC)
        # vocab range (clipped to V; pad with zero weights/bias)
        v0 = i * 6283
        nreal = VREAL[i]
        v1 = v0 + nreal

        attnp = _pack_rhs(np.ascontiguousarray(attn_WT[:, sl_l]))      # [128, 32*64]
        encp = _pack_rhs(np.ascontiguousarray(enc[:, sl_h]))           # [128, 4*256]
        comb_sel = np.concatenate(
            [comb_W[:, sl_h], comb_W[:, H + i * HC : H + (i + 1) * HC]], axis=1
        )  # [H, 512]
        combp = _pack_rhs(np.ascontiguousarray(comb_sel.T))            # [128, 4*2048]
        rows = np.concatenate(
            [np.arange(i * HC, (i + 1) * HC) + g * H for g in range(3)]
        )
        wihp = _pack_rhs(np.ascontiguousarray(w_ih[rows].T))           # [128, 16*768]
        whhp = _pack_rhs(np.ascontiguousarray(w_hh[rows].T))
        outw_sh = np.zeros((H, VC), dtype=np.float32)
        outw_sh[:, :nreal] = out_W[v0:v1].T
        outwp = np.ascontiguousarray(
            outw_sh.reshape(KH, 128, NCHUNK, VN).transpose(2, 1, 0, 3)
            .reshape(NCHUNK, 128, KH * VN)
        )
        outb_sh = np.zeros((VC,), dtype=np.float32)
        outb_sh[:nreal] = out_b[v0:v1]
        outbp = np.ascontiguousarray(outb_sh.reshape(NCHUNK, 1, VN))
        embT2 = np.ascontiguousarray(emb[sl_h].reshape(HC // 128, 128).T)

        in_maps.append({
            "cehT": cehT, "hT": hT, "hchunk": hidden[sl_h].reshape(1, HC),
            "embT2": embT2, "encp": encp, "attnp": attnp,
            "attnb": attn_b[sl_l].reshape(1, LC), "combp": combp,
            "combbT": combbT, "wihp": wihp, "whhp": whhp,
            "bih": b_ih[rows].reshape(1, 3 * HC), "bhh": b_hh[rows].reshape(1, 3 * HC),
            "outwp": outwp, "outbp": outbp, "ident": ident,
        })
    return in_maps


def _get_nc():
    if "nc" not in _CACHE:
        _CACHE["nc"] = _build()
    return _CACHE["nc"]


def kernel(**inputs):
    from concourse.bass_utils import run_bass_kernel_spmd

    in_maps = _prepare_inputs(inputs)
    nc = _get_nc()
    r = run_bass_kernel_spmd(
        nc, in_maps, core_ids=list(range(NC)), trace=TRACE,
        **({"trace_cores": list(range(NC))} if TRACE else {}),
    )
    LAST["exec_time_ns"] = r.exec_time_ns
    LAST["mean_exec_time_ns"] = r.mean_exec_time_ns
    if r.instructions_and_trace is not None:
        LAST["trace_path"] = r.instructions_and_trace[1]
    res = r.results

    logits = np.concatenate(
        [np.asarray(res[i]["o_logits"]).reshape(VC)[: VREAL[i]] for i in range(NC)]
    ).reshape(1, V)
    h_new = np.asarray(res[0]["o_hidden"]).reshape(1, 1, H)
    attn_weights = np.asarray(res[0]["o_attnw"]).reshape(1, L)
    return logits, h_new, attn_weights


# revision 24
# speedup vs baseline: 1.0815x; 1.0815x over previous
"""Trainium2 Bass kernel for nn_AttentionDecoder (single-step GRU attention
decoder, batch 1). 8-core tensor-parallel:

- attention linear sharded over L (output), logits all-gathered, softmax
  redundant on every core
- attn_applied: encoder_outputs sharded column-wise (over H)
- attn_combine sharded over the *contraction* dim -> partial [1,H] summed
  with an AllReduce, then bias+relu redundant
- GRU: w_ih / w_hh sharded over gate-output rows (each core owns its H/8
  slice of r,z,n), h_new chunks all-gathered
- out Linear sharded over vocab; per-core exp-sums all-gathered and combined
  for the log_softmax normalizer

All weight matrices are pre-transposed/packed on the host into the layout
the TensorEngine streams directly (per-partition-contiguous rhs tiles), so
every device DMA is a plain contiguous [128, F] transfer.
"""
import numpy as np

H = 2048
V = 50257
L = 512
NC = 8
HC = H // NC           # 256
LC = L // NC           # 64
KH = H // 128          # 16 k-chunks over H contraction
K2H = (2 * H) // 128   # 32 k-chunks over 2H contraction
KL = L // 128          # 4 k-chunks over L contraction
VN = 256               # vocab columns per matmul/psum chunk
NCHUNK = 25            # v-chunks per core
VC = VN * NCHUNK       # 6400 padded vocab per core
VREAL = [6283] * 7 + [V - 6283 * 7]   # real columns per core (6283/6276)
PAD_TOTAL = float(NC * VC - V)        # 943 zero-logit pad columns overall

_CACHE = {}
TRACE = False
LAST = {}


def _pack_rhs(wT):
    """[K, N] (K % 128 == 0) -> [128, (K//128)*N]; tile[p, k*N+n] = wT[k*128+p, n]."""
    K, N = wT.shape
    kk = K // 128
    return np.ascontiguousarray(
        wT.reshape(kk, 128, N).transpose(1, 0, 2).reshape(128, kk * N)
    )


def _vec_T(v):
    """[n*128] -> [128, n] with out[p, k] = v[k*128 + p]."""
    n = v.shape[0] // 128
    return np.ascontiguousarray(v.reshape(n, 128).T)


def _build():
    import concourse.bacc as bacc
    import concourse.mybir as mybir
    import concourse.tile as tile

    f32 = mybir.dt.float32
    nc = bacc.Bacc("TRN2", target_bir_lowering=False, debug=False, num_devices=NC)

    # ---- I/O ------------------------------------------------------------
    i_cehT = nc.dram_tensor("cehT", [128, K2H], f32, kind="ExternalInput")
    i_hT = nc.dram_tensor("hT", [128, KH], f32, kind="ExternalInput")
    i_hchunk = nc.dram_tensor("hchunk", [1, HC], f32, kind="ExternalInput")
    i_embT2 = nc.dram_tensor("embT2", [128, HC // 128], f32, kind="ExternalInput")
    i_enc = nc.dram_tensor("encp", [128, KL * HC], f32, kind="ExternalInput")
    i_attnw = nc.dram_tensor("attnp", [128, K2H * LC], f32, kind="ExternalInput")
    i_attnb = nc.dram_tensor("attnb", [1, LC], f32, kind="ExternalInput")
    i_comb = nc.dram_tensor("combp", [128, 4 * H], f32, kind="ExternalInput")
    i_combbT = nc.dram_tensor("combbT", [128, KH], f32, kind="ExternalInput")
    i_wih = nc.dram_tensor("wihp", [128, KH * 3 * HC], f32, kind="ExternalInput")
    i_whh = nc.dram_tensor("whhp", [128, KH * 3 * HC], f32, kind="ExternalInput")
    i_bih = nc.dram_tensor("bih", [1, 3 * HC], f32, kind="ExternalInput")
    i_bhh = nc.dram_tensor("bhh", [1, 3 * HC], f32, kind="ExternalInput")
    i_outw = nc.dram_tensor("outwp", [NCHUNK, 128, KH * VN], f32, kind="ExternalInput")
    i_outb = nc.dram_tensor("outbp", [NCHUNK, 1, VN], f32, kind="ExternalInput")
    i_ident = nc.dram_tensor("ident", [128, 128], f32, kind="ExternalInput")

    o_logits = nc.dram_tensor("o_logits", [1, VC], f32, kind="ExternalOutput")
    o_hidden = nc.dram_tensor("o_hidden", [1, H], f32, kind="ExternalOutput")
    o_attnw = nc.dram_tensor("o_attnw", [1, L], f32, kind="ExternalOutput")

    # ---- internal DRAM for collectives (all 2-D: 1-D/row DMA APs hang) --
    cc_attn_in = nc.dram_tensor("cc_attn_in", [1, LC], f32)
    cc_attn_out = nc.dram_tensor("cc_attn_out", [NC, LC], f32, addr_space="Shared")
    aw_dram = nc.dram_tensor("aw_dram", [1, L], f32)
    aa_dram = nc.dram_tensor("aa_dram", [1, HC], f32)
    cc_x_in = nc.dram_tensor("cc_x_in", [1, H], f32)
    cc_x_out = nc.dram_tensor("cc_x_out", [1, H], f32, addr_space="Shared")
    cc_h_in = nc.dram_tensor("cc_h_in", [1, HC], f32)
    cc_h_out = nc.dram_tensor("cc_h_out", [NC, HC], f32, addr_space="Shared")
    cc_s_in = nc.dram_tensor("cc_s_in", [1, 1], f32)
    cc_s_out = nc.dram_tensor("cc_s_out", [NC, 1], f32, addr_space="Shared")

    AF = mybir.ActivationFunctionType
    ALU = mybir.AluOpType
    rg = [list(range(NC))]

    with tile.TileContext(nc) as tc:
        with (
            tc.tile_pool(name="const", bufs=1) as cpool,
            tc.tile_pool(name="wts", bufs=1) as wpool,
            tc.tile_pool(name="gruw", bufs=2) as gpool,
            tc.tile_pool(name="stream", bufs=4) as spool,
            tc.tile_pool(name="small", bufs=2) as smpool,
            tc.tile_pool(name="ps", bufs=1, space="PSUM") as ps,
        ):
            # ---------- constant / small loads ----------
            ident = cpool.tile([128, 128], f32, tag="ident")
            nc.sync.dma_start(ident[:], i_ident[:])
            cehT = cpool.tile([128, K2H], f32, tag="cehT")
            nc.sync.dma_start(cehT[:], i_cehT[:])
            hT = cpool.tile([128, KH], f32, tag="hT")
            nc.sync.dma_start(hT[:], i_hT[:])
            embT2 = cpool.tile([128, HC // 128], f32, tag="embT2")
            nc.sync.dma_start(embT2[:], i_embT2[:])
            combbT = cpool.tile([128, KH], f32, tag="combbT")
            nc.sync.dma_start(combbT[:], i_combbT[:])
            hchunk = cpool.tile([1, HC], f32, tag="hchunk")
            nc.sync.dma_start(hchunk[:], i_hchunk[:])
            attnb = cpool.tile([1, LC], f32, tag="attnb")
            nc.sync.dma_start(attnb[:], i_attnb[:])
            bih = cpool.tile([1, 3 * HC], f32, tag="bih")
            nc.sync.dma_start(bih[:], i_bih[:])
            bhh = cpool.tile([1, 3 * HC], f32, tag="bhh")
            nc.sync.dma_start(bhh[:], i_bhh[:])

            # ---------- stage A: attention logits + AG + softmax ----------
            attnp = wpool.tile([128, K2H * LC], f32, tag="attnp")
            nc.sync.dma_start(attnp[:], i_attnw[:])
            p_attn = ps.tile([1, 512], f32, tag="big", bufs=3)
            for k in range(K2H):
                nc.tensor.matmul(
                    p_attn[0:1, 0:LC],
                    cehT[:, k : k + 1],
                    attnp[:, k * LC : (k + 1) * LC],
                    start=(k == 0),
                    stop=(k == K2H - 1),
                )
            al_c = cpool.tile([1, LC], f32, tag="al_c")
            nc.vector.tensor_add(al_c[:], p_attn[0:1, 0:LC], attnb[:])
            nc.sync.dma_start(cc_attn_in[:], al_c[:])
            nc.gpsimd.collective_compute(
                "AllGather", ALU.bypass, replica_groups=rg,
                ins=[cc_attn_in[:].opt()], outs=[cc_attn_out[:].opt()],
            )
            al_full = cpool.tile([1, L], f32, tag="al_full")
            nc.sync.dma_start(al_full[:], cc_attn_out[:].rearrange("a b -> (a b)")[None, :])
            # softmax over [1, L] (logits are O(1): no max subtraction needed)
            s_aw = cpool.tile([1, 1], f32, tag="s_aw")
            aw = cpool.tile([1, L], f32, tag="aw")
            nc.scalar.activation(aw[:], al_full[:], AF.Exp, accum_out=s_aw[:])
            sinv = cpool.tile([1, 1], f32, tag="sinv")
            nc.vector.reciprocal(sinv[:], s_aw[:])
            nc.vector.tensor_scalar_mul(aw[:], aw[:], sinv[0:1, 0:1])
            nc.sync.dma_start(o_attnw[:], aw[:])
            # aw -> lhsT [128, KL] via dram roundtrip + PE transpose
            nc.sync.dma_start(aw_dram[:], aw[:])
            awKL = cpool.tile([KL, 128], f32, tag="awKL")
            nc.sync.dma_start(awKL[:], aw_dram[:].rearrange("a (b c) -> (a b) c", b=KL))
            p_awT = ps.tile([128, KL], f32, tag="tr")
            nc.tensor.transpose(p_awT[:], awKL[:], ident[0:KL, 0:KL])
            awT = cpool.tile([128, KL], f32, tag="awT")
            nc.vector.tensor_copy(awT[:], p_awT[:])

            # ---------- stage B: attn_applied chunk + comb lhsT ----------
            encp = wpool.tile([128, KL * HC], f32, tag="encp")
            nc.sync.dma_start(encp[:], i_enc[:])
            p_aa = ps.tile([1, 512], f32, tag="big", bufs=3)
            for k in range(KL):
                nc.tensor.matmul(
                    p_aa[0:1, 0:HC],
                    awT[:, k : k + 1],
                    encp[:, k * HC : (k + 1) * HC],
                    start=(k == 0),
                    stop=(k == KL - 1),
                )
            aa = cpool.tile([1, HC], f32, tag="aa")
            nc.vector.tensor_copy(aa[:], p_aa[0:1, 0:HC])
            nc.sync.dma_start(aa_dram[:], aa[:])
            aa2 = cpool.tile([HC // 128, 128], f32, tag="aa2")
            nc.sync.dma_start(
                aa2[:], aa_dram[:].rearrange("a (b c) -> (a b) c", b=HC // 128)
            )
            comb_in_T = cpool.tile([128, 2 * (HC // 128)], f32, tag="comb_in_T")
            p_aaT = ps.tile([128, HC // 128], f32, tag="tr")
            nc.tensor.transpose(p_aaT[:], aa2[:], ident[0 : HC // 128, 0 : HC // 128])
            nc.vector.tensor_copy(comb_in_T[:, 0 : HC // 128], embT2[:])
            nc.vector.tensor_copy(comb_in_T[:, HC // 128 :], p_aaT[:])

            # ---------- stage C: comb partial + AllReduce + relu ----------
            combp = wpool.tile([128, 4 * H], f32, tag="combp")
            nc.sync.dma_start(combp[:], i_comb[:])
            x_part = cpool.tile([1, H], f32, tag="x_part")
            KC = 2 * HC // 128  # 4 k-chunks over the 512-dim contraction
            for q in range(H // 512):   # four 512-wide output slices
                p_x = ps.tile([1, 512], f32, tag="big", bufs=3)
                for k in range(KC):
                    nc.tensor.matmul(
                        p_x[:],
                        comb_in_T[:, k : k + 1],
                        combp[:, k * H + q * 512 : k * H + (q + 1) * 512],
                        start=(k == 0),
                        stop=(k == KC - 1),
                    )
                nc.vector.tensor_copy(x_part[0:1, q * 512 : (q + 1) * 512], p_x[:])
            nc.sync.dma_start(cc_x_in[:], x_part[:])
            nc.gpsimd.collective_compute(
                "AllReduce", ALU.add, replica_groups=rg,
                ins=[cc_x_in[:].opt()], outs=[cc_x_out[:].opt()],
            )
            x16 = cpool.tile([KH, 128], f32, tag="x16")
            nc.sync.dma_start(
                x16[:], cc_x_out[:].rearrange("a (b c) -> (a b) c", b=KH)
            )
            p_xT = ps.tile([128, KH], f32, tag="tr")
            nc.tensor.transpose(p_xT[:], x16[:], ident[0:KH, 0:KH])
            xsum = cpool.tile([128, KH], f32, tag="xsum")
            nc.vector.tensor_add(xsum[:], p_xT[:], combbT[:])
            xT = cpool.tile([128, KH], f32, tag="xT")
            nc.scalar.activation(xT[:], xsum[:], AF.Relu)

            # ---------- stage E: GRU gates ----------
            G = 3 * HC  # 768
            p_gi = ps.tile([1, G], f32, tag="gi")
            p_gh = ps.tile([1, G], f32, tag="gh")
            quarters = [(i * (KH // 4), (i + 1) * (KH // 4)) for i in range(4)]
            for (name, pk, lhsT, w_in) in (
                ("hh", p_gh, hT, i_whh),
                ("ih", p_gi, xT, i_wih),
            ):
                for qi, (h0, h1) in enumerate(quarters):
                    wt = gpool.tile([128, (KH // 4) * G], f32, tag="gruw")
                    eng = nc.sync if qi % 2 == 0 else nc.scalar
                    eng.dma_start(wt[:], w_in[:, h0 * G : h1 * G])
                    for k in range(h0, h1):
                        off = (k - h0) * G
                        for n0, n1 in ((0, 512), (512, G)):
                            nc.tensor.matmul(
                                pk[0:1, n0:n1],
                                lhsT[:, k : k + 1],
                                wt[:, off + n0 : off + n1],
                                start=(k == 0),
                                stop=(k == KH - 1),
                            )
            gi = cpool.tile([1, G], f32, tag="gi")
            nc.vector.tensor_add(gi[:], p_gi[:], bih[:])
            gh = cpool.tile([1, G], f32, tag="gh")
            nc.vector.tensor_add(gh[:], p_gh[:], bhh[:])
            t_r = cpool.tile([1, HC], f32, tag="t_r")
            nc.vector.tensor_add(t_r[:], gi[0:1, 0:HC], gh[0:1, 0:HC])
            r_g = cpool.tile([1, HC], f32, tag="r_g")
            nc.scalar.activation(r_g[:], t_r[:], AF.Sigmoid)
            t_z = cpool.tile([1, HC], f32, tag="t_z")
            nc.vector.tensor_add(t_z[:], gi[0:1, HC : 2 * HC], gh[0:1, HC : 2 * HC])
            z_g = cpool.tile([1, HC], f32, tag="z_g")
            nc.scalar.activation(z_g[:], t_z[:], AF.Sigmoid)
            t_n = cpool.tile([1, HC], f32, tag="t_n")
            nc.vector.tensor_mul(t_n[:], r_g[:], gh[0:1, 2 * HC : 3 * HC])
            t_n2 = cpool.tile([1, HC], f32, tag="t_n2")
            nc.vector.tensor_add(t_n2[:], t_n[:], gi[0:1, 2 * HC : 3 * HC])
            n_g = cpool.tile([1, HC], f32, tag="n_g")
            nc.scalar.activation(n_g[:], t_n2[:], AF.Tanh)
            # h_new = n + z * (h - n)
            t_d = cpool.tile([1, HC], f32, tag="t_d")
            nc.vector.tensor_sub(t_d[:], hchunk[:], n_g[:])
            t_e = cpool.tile([1, HC], f32, tag="t_e")
            nc.vector.tensor_mul(t_e[:], z_g[:], t_d[:])
            hn = cpool.tile([1, HC], f32, tag="hn")
            nc.vector.tensor_add(hn[:], n_g[:], t_e[:])
            nc.sync.dma_start(cc_h_in[:], hn[:])
            nc.gpsimd.collective_compute(
                "AllGather", ALU.bypass, replica_groups=rg,
                ins=[cc_h_in[:].opt()], outs=[cc_h_out[:].opt()],
            )
            nc.sync.dma_start(
                o_hidden[:], cc_h_out[:].rearrange("a b -> (a b)")[None, :]
            )
            h16 = cpool.tile([KH, 128], f32, tag="h16")
            nc.sync.dma_start(
                h16[:], cc_h_out[:].rearrange("a (b c) -> (a b) c", b=KH // NC)
            )
            p_hT = ps.tile([128, KH], f32, tag="tr")
            nc.tensor.transpose(p_hT[:], h16[:], ident[0:KH, 0:KH])
            hnT = cpool.tile([128, KH], f32, tag="hnT")
            nc.vector.tensor_copy(hnT[:], p_hT[:])

            # ---------- stage F: vocab GEMV stream ----------
            logits = cpool.tile([1, VC], f32, tag="logits")
            sums = cpool.tile([1, NCHUNK], f32, tag="sums")
            expscr = cpool.tile([1, VN], f32, tag="expscr")
            HW2 = KH * VN // 2
            for c in range(NCHUNK):
                wt = spool.tile([128, KH * VN], f32, tag="outw")
                # two half-transfers on the two HWDGE rings: overlaps the
                # per-DMA fixed cost and lets k<8 matmuls start at half-arrival
                nc.sync.dma_start(wt[:, 0:HW2], i_outw[c, :, 0:HW2])
                nc.scalar.dma_start(wt[:, HW2:], i_outw[c, :, HW2:])
                ob = smpool.tile([1, VN], f32, tag="outb")
                nc.sync.dma_start(ob[:], i_outb[c])
                p_l = ps.tile([1, VN], f32, tag="big", bufs=3)
                for k in range(KH):
                    nc.tensor.matmul(
                        p_l[:],
                        hnT[:, k : k + 1],
                        wt[:, k * VN : (k + 1) * VN],
                        start=(k == 0),
                        stop=(k == KH - 1),
                    )
                nc.vector.tensor_add(logits[0:1, c * VN : (c + 1) * VN], p_l[:], ob[:])
                nc.scalar.activation(
                    expscr[:],
                    logits[0:1, c * VN : (c + 1) * VN],
                    AF.Exp,
                    accum_out=sums[0:1, c : c + 1],
                )

            # ---------- normalizer ----------
            s_loc = cpool.tile([1, 1], f32, tag="s_loc")
            nc.vector.reduce_sum(s_loc[:], sums[:], axis=mybir.AxisListType.X)
            nc.sync.dma_start(cc_s_in[:], s_loc[:])
            nc.gpsimd.collective_compute(
                "AllGather", ALU.bypass, replica_groups=rg,
                ins=[cc_s_in[:].opt()], outs=[cc_s_out[:].opt()],
            )
            s8 = cpool.tile([1, NC], f32, tag="s8")
            nc.sync.dma_start(s8[:], cc_s_out[:].rearrange("a b -> (a b)")[None, :])
            s_tot = cpool.tile([1, 1], f32, tag="s_tot")
            nc.vector.reduce_sum(s_tot[:], s8[:], axis=mybir.AxisListType.X)
            s_corr = cpool.tile([1, 1], f32, tag="s_corr")
            nc.vector.tensor_scalar_add(s_corr[:], s_tot[:], -PAD_TOTAL)
            logS = cpool.tile([1, 1], f32, tag="logS")
            nc.scalar.activation(logS[:], s_corr[:], AF.Ln)
            nc.vector.tensor_scalar(
                logits[:], logits[:], logS[0:1, 0:1], None, ALU.subtract
            )
            nc.sync.dma_start(o_logits[:], logits[:])

    nc.compile()
    return nc


def _prepare_inputs(inputs):
    token = int(np.asarray(inputs["token"]).ravel()[0])
    hidden = np.asarray(inputs["hidden"], dtype=np.float32).reshape(H)
    enc = np.asarray(inputs["encoder_outputs"], dtype=np.float32)
    emb = np.asarray(inputs["embedding"], dtype=np.float32)[token]
    attn_W = np.asarray(inputs["attn_W"], dtype=np.float32)
    attn_b = np.asarray(inputs["attn_b"], dtype=np.float32)
    comb_W = np.asarray(inputs["comb_W"], dtype=np.float32)
    comb_b = np.asarray(inputs["comb_b"], dtype=np.float32)
    w_ih = np.asarray(inputs["w_ih"], dtype=np.float32)
    w_hh = np.asarray(inputs["w_hh"], dtype=np.float32)
    b_ih = np.asarray(inputs["b_ih"], dtype=np.float32)
    b_hh = np.asarray(inputs["b_hh"], dtype=np.float32)
    out_W = np.asarray(inputs["out_W"], dtype=np.float32)
    out_b = np.asarray(inputs["out_b"], dtype=np.float32)

    ceh = np.concatenate([emb, hidden])            # [2H]
    cehT = _vec_T(ceh)                             # [128, 32]
    hT = _vec_T(hidden)                            # [128, 16]
    combbT = _vec_T(comb_b)                        # [128, 16]
    ident = np.eye(128, dtype=np.float32)
    attn_WT = np.ascontiguousarray(attn_W.T)       # [2H, L]

    in_maps = []
    for i in range(NC):
        sl_h = slice(i * HC, (i + 1) * HC)
        sl_l = slice(i * LC, (i + 1) * LC)
        # vocab range (clipped to V; pad with zero weights/bias)
        v0 = i * 6283
        nreal = VREAL[i]
        v1 = v0 + nreal

        attnp = _pack_rhs(np.ascontiguousarray(attn_WT[:, sl_l]))      # [128, 32*64]
        encp = _pack_rhs(np.ascontiguousarray(enc[:, sl_h]))           # [128, 4*256]
        comb_sel = np.concatenate(
            [comb_W[:, sl_h], comb_W[:, H + i * HC : H + (i + 1) * HC]], axis=1
        )  # [H, 512]
        combp = _pack_rhs(np.ascontiguousarray(comb_sel.T))            # [128, 4*2048]
        rows = np.concatenate(
            [np.arange(i * HC, (i + 1) * HC) + g * H for g in range(3)]
        )
        wihp = _pack_rhs(np.ascontiguousarray(w_ih[rows].T))           # [128, 16*768]
        whhp = _pack_rhs(np.ascontiguousarray(w_hh[rows].T))
        outw_sh = np.zeros((H, VC), dtype=np.float32)
        outw_sh[:, :nreal] = out_W[v0:v1].T
        outwp = np.ascontiguousarray(
            outw_sh.reshape(KH, 128, NCHUNK, VN).transpose(2, 1, 0, 3)
            .reshape(NCHUNK, 128, KH * VN)
        )
        outb_sh = np.zeros((VC,), dtype=np.float32)
        outb_sh[:nreal] = out_b[v0:v1]
        outbp = np.ascontiguousarray(outb_sh.reshape(NCHUNK, 1, VN))
        embT2 = np.ascontiguousarray(emb[sl_h].reshape(HC // 128, 128).T)

        in_maps.append({
            "cehT": cehT, "hT": hT, "hchunk": hidden[sl_h].reshape(1, HC),
            "embT2": embT2, "encp": encp, "attnp": attnp,
            "attnb": attn_b[sl_l].reshape(1, LC), "combp": combp,
            "combbT": combbT, "wihp": wihp, "whhp": whhp,
            "bih": b_ih[rows].reshape(1, 3 * HC), "bhh": b_hh[rows].reshape(1, 3 * HC),
            "outwp": outwp, "outbp": outbp, "ident": ident,
        })
    return in_maps


def _get_nc():
    if "nc" not in _CACHE:
        _CACHE["nc"] = _build()
    return _CACHE["nc"]


def kernel(**inputs):
    from concourse.bass_utils import run_bass_kernel_spmd

    in_maps = _prepare_inputs(inputs)
    nc = _get_nc()
    r = run_bass_kernel_spmd(
        nc, in_maps, core_ids=list(range(NC)), trace=TRACE,
        **({"trace_cores": list(range(NC))} if TRACE else {}),
    )
    LAST["exec_time_ns"] = r.exec_time_ns
    LAST["mean_exec_time_ns"] = r.mean_exec_time_ns
    if r.instructions_and_trace is not None:
        LAST["trace_path"] = r.instructions_and_trace[1]
    res = r.results

    logits = np.concatenate(
        [np.asarray(res[i]["o_logits"]).reshape(VC)[: VREAL[i]] for i in range(NC)]
    ).reshape(1, V)
    h_new = np.asarray(res[0]["o_hidden"]).reshape(1, 1, H)
    attn_weights = np.asarray(res[0]["o_attnw"]).reshape(1, L)
    return logits, h_new, attn_weights
